# revision 3
# baseline (speedup 1.0000x reference)
"""Causal self-attention Bass kernel for 8 TRN2 NeuronCores.

Problem: B=4, T=2048, C=1024, H=16 heads, head_dim=64, fp32.
    q = x @ Wq.T ; k = x @ Wk.T ; v = x @ Wv.T          (per head)
    att = softmax(mask(q k^T / 8))
    y = att @ v ; out = y @ Wp.T

Sharding (8 cores): 4-way data parallel over batch x 2-way tensor
parallel over heads. Core c handles batch c//2 and heads 8*(c%2)..+8.
Wq/Wk/Wv column-parallel, Wp row-parallel; the partial outputs of the
two head-halves of each batch are summed on the host (the "all-reduce"
of row-parallel Wp).

Single fused pipeline (no phase barriers): the attention loop runs
qc-outer (q chunks of 512) / head-pair inner / k-tile innermost, and a
filler queue interleaves projection + output-projection matmuls into
the PE bubbles left while ScalarE computes exp.  Everything is bf16
(host-pretransposed and pre-cast), which keeps weight loads on the
fast path and halves input DMA.  Scores for the two heads of a pair
run CONCURRENTLY on PE row-groups (K=64 contraction, lhsT at base
partitions 0/64 -> auto tile_position (0,0)/(64,0)).  Softmax
normalization is moved off the critical path: PSUM y is quick-released
via a copy, then reciprocal (DVE) -> partition_broadcast + multiply
(GpSimd) -> repartition DMA into yT_all.
"""

from contextlib import ExitStack

import numpy as np

import concourse.bass as bass
import concourse.tile as tile
from concourse import bacc, mybir

F32 = mybir.dt.float32
BF16 = mybir.dt.bfloat16

B, T, C, H, D = 4, 2048, 1024, 16, 64
NCORES = 8
JL = 512            # local j dims per core (8 heads * 64)
NPAIR = 4           # local head pairs
CI = C // 128       # 8 c-tiles
NT = T // 128       # 16 t/k tiles
NQC = T // 512      # 4 q chunks
VW = D + 1          # ones column + head dim

_CACHED_NC = None


def build_nc():
    nc = bacc.Bacc(None)

    xT = nc.dram_tensor("xT", [C, T], BF16, kind="ExternalInput")
    wqT = nc.dram_tensor("wqT", [C, JL], BF16, kind="ExternalInput")
    wkT = nc.dram_tensor("wkT", [C, JL], BF16, kind="ExternalInput")
    wvT = nc.dram_tensor("wvT", [C, JL], BF16, kind="ExternalInput")
    wpT = nc.dram_tensor("wpT", [JL, C], BF16, kind="ExternalInput")
    out = nc.dram_tensor("out", [T, C], F32, kind="ExternalOutput")

    xT_r = xT.rearrange("(ci p) t -> p ci t", p=128)
    wq_r = wqT.rearrange("(ci p) j -> p ci j", p=128)
    wk_r = wkT.rearrange("(ci p) j -> p ci j", p=128)
    wv_r = wvT.rearrange("(ci p) j -> p ci j", p=128)
    wp_r = wpT.rearrange("(ji p) c -> p ji c", p=128)

    with tile.TileContext(nc) as tc, ExitStack() as ctx:
        pm = ctx.enter_context(tc.tile_pool(name="pm", bufs=1))
        qkp = ctx.enter_context(tc.tile_pool(name="qkp", bufs=1))
        expp = ctx.enter_context(tc.tile_pool(name="expp", bufs=3))
        cpp = ctx.enter_context(tc.tile_pool(name="cpp", bufs=2))
        rcp = ctx.enter_context(tc.tile_pool(name="rcp", bufs=2))
        bcp = ctx.enter_context(tc.tile_pool(name="bcp", bufs=2))
        stp = ctx.enter_context(tc.tile_pool(name="stp", bufs=2))
        outp = ctx.enter_context(tc.tile_pool(name="outp", bufs=3))
        # PSUM budget (8 banks): fillp 2 + gp 4 + yp 2
        fillp = ctx.enter_context(tc.tile_pool(name="fillp", bufs=2, space="PSUM"))
        gp = ctx.enter_context(tc.tile_pool(name="gp", bufs=2, space="PSUM"))
        yp = ctx.enter_context(tc.tile_pool(name="yp", bufs=2, space="PSUM"))

        x_sb = pm.tile([128, CI, T], BF16, tag="x")
        wq_sb = pm.tile([128, CI, JL], BF16, tag="wq")
        wk_sb = pm.tile([128, CI, JL], BF16, tag="wk")
        wv_sb = pm.tile([128, CI, JL], BF16, tag="wv")
        wp_sb = pm.tile([128, NPAIR, C], BF16, tag="wp")
        # v with a ones column prepended per head (softmax sums land on psum
        # partition 0) plus 64 pad columns so every per-head lhsT reads as
        # [128, 128] (fast weight load path).
        v_sb = pm.tile([128, NT, 8 * VW + 64], BF16, tag="v")
        v_view = v_sb[:, :, 0 : 8 * VW].rearrange("p n (h w) -> p n h w", w=VW)
        qT_all = qkp.tile([128, NPAIR, T], BF16, tag="qT")
        kT_all = qkp.tile([128, NPAIR, T], BF16, tag="kT")
        yT_all = qkp.tile([128, NPAIR, T], BF16, tag="yT")

        ones_col = pm.tile([128, NT, 8, 1], F32)
        nc.vector.memset(ones_col[:], 1.0)
        nc.vector.tensor_copy(v_view[:, :, :, 0:1], ones_col[:])
        nc.vector.memset(v_sb[:, :, 8 * VW : 8 * VW + 64], 0.0)

        # ---- input DMAs, priority order (first q/k chains unblock first) --
        for ci in range(CI):
            nc.sync.dma_start(wq_sb[:, ci, :], wq_r[:, ci, :])
            nc.sync.dma_start(wk_sb[:, ci, :], wk_r[:, ci, :])
            nc.sync.dma_start(x_sb[:, ci, 0:512], xT_r[:, ci, 0:512])
        for ci in range(CI):
            nc.sync.dma_start(wv_sb[:, ci, :], wv_r[:, ci, :])
            nc.sync.dma_start(x_sb[:, ci, 512:1024], xT_r[:, ci, 512:1024])
        nc.sync.dma_start(wp_sb[:], wp_r[:])
        for tch in (2, 3):
            ts = slice(tch * 512, tch * 512 + 512)
            for ci in range(CI):
                nc.sync.dma_start(x_sb[:, ci, ts], xT_r[:, ci, ts])

        # ---- filler machinery: proj/outproj matmuls pad attention gaps ----
        fill_steps = []
        tag_end = {}
        cursor = [0]

        def _consume_to(end):
            end = min(end, len(fill_steps))
            while cursor[0] < end:
                fill_steps[cursor[0]]()
                cursor[0] += 1

        def consume(n):
            _consume_to(cursor[0] + n)

        def consume_until(tag):
            if tag in tag_end:
                _consume_to(tag_end[tag])

        def avail():
            return len(fill_steps) - cursor[0]

        def add_qk_unit(pr, tch):
            ts = slice(tch * 512, tch * 512 + 512)
            for w_sb, dst in ((wq_sb, qT_all), (wk_sb, kT_all)):
                box = {}
                for ci in range(CI):
                    def mm(ci=ci, w_sb=w_sb, box=box):
                        if ci == 0:
                            box["acc"] = fillp.tile([128, 512], F32, tag="fill", name="facc")
                        nc.tensor.matmul(
                            box["acc"][:],
                            w_sb[:, ci, pr * 128 : pr * 128 + 128],
                            x_sb[:, ci, ts],
                            start=(ci == 0),
                            stop=(ci == CI - 1),
                        )
                    fill_steps.append(mm)
                def cp(dst=dst, box=box):
                    nc.vector.tensor_copy(dst[:, pr, ts], box["acc"][:])
                fill_steps.append(cp)
            tag_end[("qk", pr, tch)] = len(fill_steps)

        def add_v_unit(ti):
            tch, tl = divmod(ti, 4)
            base = tch * 512 + tl * 128
            box = {}
            for ci in range(CI):
                def mm(ci=ci, box=box):
                    if ci == 0:
                        box["acc"] = fillp.tile([128, 512], F32, tag="fill", name="facc")
                    nc.tensor.matmul(
                        box["acc"][:],
                        x_sb[:, ci, base : base + 128],
                        wv_sb[:, ci, :],
                        start=(ci == 0),
                        stop=(ci == CI - 1),
                    )
                fill_steps.append(mm)
            def cp(box=box, ti=ti):
                nc.vector.tensor_copy(
                    v_view[:, ti, :, 1:VW],
                    box["acc"][:].rearrange("p (h d) -> p h d", d=D),
                )
            fill_steps.append(cp)
            tag_end[("v", ti)] = len(fill_steps)

        def add_op_unit(ti, cc):
            tss = slice(ti * 128, ti * 128 + 128)
            cs = slice(cc * 512, cc * 512 + 512)
            box = {}
            for ji in range(NPAIR):
                def mm(ji=ji, box=box):
                    if ji == 0:
                        box["acc"] = fillp.tile([128, 512], F32, tag="fill", name="facc")
                    nc.tensor.matmul(
                        box["acc"][:],
                        yT_all[:, ji, tss],
                        wp_sb[:, ji, cs],
                        start=(ji == 0),
                        stop=(ji == NPAIR - 1),
                    )
                fill_steps.append(mm)
            def cpdma(box=box):
                o = outp.tile([128, 512], F32, tag="o")
                nc.vector.tensor_copy(o[:], box["acc"][:])
                nc.sync.dma_start(out[tss, cs], o[:])
            fill_steps.append(cpdma)

        # upfront: deps of the very first attention block
        add_qk_unit(0, 0)
        add_v_unit(0)
        consume(1 << 30)
        for ti in (1, 2, 3):
            add_v_unit(ti)
        for pr in (1, 2, 3):
            add_qk_unit(pr, 0)

        # pace fillers over a horizon of (rest of this round + next round)
        horizon = [0]

        def pace():
            if horizon[0] > 0 and avail() > 0:
                consume(-(-avail() // horizon[0]))
            horizon[0] -= 1

        round_kts = [NPAIR * (4 * qc + 4) for qc in range(NQC)]

        for qc in range(NQC):
            if qc < NQC - 1:
                add_qk_unit(0, qc + 1)
                add_qk_unit(1, qc + 1)
                for ti in range(4 * qc + 4, 4 * qc + 8):
                    add_v_unit(ti)
                add_qk_unit(2, qc + 1)
                add_qk_unit(3, qc + 1)
            nkt = 4 * qc + 4
            horizon[0] = round_kts[qc] + (
                round_kts[qc + 1] if qc + 1 < NQC else 0
            )
            for pr in range(NPAIR):
                consume_until(("qk", pr, qc))
                qs = slice(qc * 512, qc * 512 + 512)
                qlo = qT_all[0:64, pr, :]
                qhi = qT_all[64:128, pr, :]
                klo = kT_all[0:64, pr, :]
                khi = kT_all[64:128, pr, :]
                yA = yp.tile([128, 512], F32, tag="y")
                yB = yp.tile([128, 512], F32, tag="y")

                def emit_pv(kt, e, yA=yA, yB=yB, pr=pr, qc=qc, nkt=nkt):
                    dt = kt - 4 * qc
                    lo = dt * 128 if dt > 0 else 0
                    nc.tensor.matmul(
                        yA[:, lo:512],
                        v_sb[:, kt, 2 * pr * VW : 2 * pr * VW + 128],
                        e[:, 0, lo:512],
                        start=(kt == 0),
                        stop=(kt == nkt - 1),
                    )
                    nc.tensor.matmul(
                        yB[:, lo:512],
                        v_sb[:, kt, (2 * pr + 1) * VW : (2 * pr + 1) * VW + 128],
                        e[:, 1, lo:512],
                        start=(kt == 0),
                        stop=(kt == nkt - 1),
                    )

                prev = None
                for kt in range(nkt):
                    pace()
                    dt = kt - 4 * qc
                    xlo = dt * 128 if dt > 0 else 0
                    ks = slice(kt * 128, kt * 128 + 128)
                    qsx = slice(qc * 512 + xlo, qc * 512 + 512)
                    g = gp.tile([128, 2, 512], F32, tag="g")
                    # the two heads' score matmuls run concurrently on PE
                    # row-groups (K=64 at base partitions 0 and 64)
                    nc.tensor.matmul(
                        g[:, 0, xlo:512], klo[:, ks], qlo[:, qsx],
                        start=True, stop=True,
                    )
                    nc.tensor.matmul(
                        g[:, 1, xlo:512], khi[:, ks], qhi[:, qsx],
                        start=True, stop=True,
                    )
                    e = expp.tile([128, 2, 512], BF16, tag="e")
                    nc.scalar.activation(
                        e[:, :, xlo:512],
                        g[:, :, xlo:512],
                        mybir.ActivationFunctionType.Exp,
                        scale=0.125,
                    )
                    if dt >= 0:
                        # zero the causal triangle (k > q) of the diagonal
                        # block on the otherwise-idle gpsimd engine
                        bs = slice(dt * 128, dt * 128 + 128)
                        for h in (0, 1):
                            nc.gpsimd.affine_select(
                                out=e[:, h, bs],
                                in_=e[:, h, bs],
                                compare_op=mybir.AluOpType.is_ge,
                                fill=0.0,
                                base=0,
                                pattern=[[1, 128]],
                                channel_multiplier=-1,
                            )
                    if prev is not None:
                        consume_until(("v", prev[0]))
                        emit_pv(*prev)
                    prev = (kt, e)
                consume_until(("v", prev[0]))
                emit_pv(*prev)

                # quick-release PSUM, then normalize off the critical path
                cp = cpp.tile([VW, 2, 512], F32, tag="cp")
                nc.vector.tensor_copy(cp[:, 0, :], yA[0:VW, :])
                nc.vector.tensor_copy(cp[:, 1, :], yB[0:VW, :])
                rc = rcp.tile([1, 2, 512], F32, tag="rc")
                nc.vector.reciprocal_approx_fast(rc[0:1, 0, :], cp[0:1, 0, :])
                nc.vector.reciprocal_approx_fast(rc[0:1, 1, :], cp[0:1, 1, :])
                bc = bcp.tile([VW, 2, 512], F32, tag="bc")
                nc.gpsimd.partition_broadcast(bc[:], rc[0:1, :, :])
                stg = stp.tile([VW, 2, 512], BF16, tag="stg")
                nc.gpsimd.tensor_mul(stg[:], cp[:], bc[:])
                nc.sync.dma_start(yT_all[0:64, pr, qs], stg[1:VW, 0, :])
                nc.sync.dma_start(yT_all[64:128, pr, qs], stg[1:VW, 1, :])
            # this round's outputs become next round's filler work
            for ti in range(4 * qc, 4 * qc + 4):
                for cc in range(2):
                    add_op_unit(ti, cc)

        consume(1 << 30)

    nc.finalize()
    return nc


def _get_nc():
    global _CACHED_NC
    if _CACHED_NC is None:
        _CACHED_NC = build_nc()
    return _CACHED_NC


def kernel(x, Wq, Wk, Wv, Wp):
    import ml_dtypes
    from concourse.bass_utils import run_bass_kernel_spmd

    bf16 = ml_dtypes.bfloat16
    x = np.asarray(x, dtype=np.float32)
    Wq = np.asarray(Wq, dtype=np.float32)
    Wk = np.asarray(Wk, dtype=np.float32)
    Wv = np.asarray(Wv, dtype=np.float32)
    Wp = np.asarray(Wp, dtype=np.float32)

    nc = _get_nc()

    xT = [np.ascontiguousarray(x[b].T).astype(bf16) for b in range(B)]
    wqT, wkT, wvT, wpT = [], [], [], []
    for hh in range(2):
        js = slice(JL * hh, JL * hh + JL)
        wqT.append(np.ascontiguousarray(Wq[js, :].T).astype(bf16))
        wkT.append(np.ascontiguousarray(Wk[js, :].T).astype(bf16))
        wvT.append(np.ascontiguousarray(Wv[js, :].T).astype(bf16))
        wpT.append(np.ascontiguousarray(Wp[:, js].T).astype(bf16))

    in_maps = []
    for c in range(NCORES):
        b, hh = c // 2, c % 2
        in_maps.append(
            {
                "xT": xT[b],
                "wqT": wqT[hh],
                "wkT": wkT[hh],
                "wvT": wvT[hh],
                "wpT": wpT[hh],
            }
        )

    res = run_bass_kernel_spmd(nc, in_maps, core_ids=list(range(NCORES)))

    out = np.empty((B, T, C), dtype=np.float32)
    for b in range(B):
        out[b] = res.results[2 * b]["out"] + res.results[2 * b + 1]["out"]
    return out


# revision 6
# speedup vs baseline: 1.6158x; 1.6158x over previous
"""Causal self-attention Bass kernel for 8 TRN2 NeuronCores.

Problem: B=4, T=2048, C=1024, H=16 heads, head_dim=64, fp32.
    q = x @ Wq.T ; k = x @ Wk.T ; v = x @ Wv.T          (per head)
    att = softmax(mask(q k^T / 8))
    y = att @ v ; out = y @ Wp.T

Sharding (8 cores): 4-way data parallel over batch x 2-way tensor
parallel over heads. Core c handles batch c//2 and heads 8*(c%2)..+8.
Wq/Wk/Wv column-parallel, Wp row-parallel; the partial outputs of the
two head-halves of each batch are summed on the host (the "all-reduce"
of row-parallel Wp).

Single fused pipeline (no phase barriers): the attention loop runs
qc-outer (q chunks of 512) / head-pair inner / k-tile innermost, and a
filler queue interleaves projection + output-projection matmuls into
the PE bubbles left while ScalarE computes exp.  Everything is bf16
(host-pretransposed and pre-cast), which keeps weight loads on the
fast path and halves input DMA.  Scores for the two heads of a pair
run CONCURRENTLY on PE row-groups (K=64 contraction, lhsT at base
partitions 0/64 -> auto tile_position (0,0)/(64,0)).  Softmax
normalization is moved off the critical path: PSUM y is quick-released
via a copy, then reciprocal (DVE) -> partition_broadcast + multiply
(GpSimd) -> repartition DMA into yT_all.
"""

from contextlib import ExitStack

import numpy as np

import concourse.bass as bass
import concourse.tile as tile
from concourse import bacc, mybir

F32 = mybir.dt.float32
BF16 = mybir.dt.bfloat16

B, T, C, H, D = 4, 2048, 1024, 16, 64
NCORES = 8
JL = 512            # local j dims per core (8 heads * 64)
NPAIR = 4           # local head pairs
CI = C // 128       # 8 c-tiles
NT = T // 128       # 16 t/k tiles
NQC = T // 512      # 4 q chunks
VW = D + 1          # ones column + head dim

_CACHED_NC = None


def build_nc():
    nc = bacc.Bacc(None)

    xT = nc.dram_tensor("xT", [C, T], BF16, kind="ExternalInput")
    wqT = nc.dram_tensor("wqT", [C, JL], BF16, kind="ExternalInput")
    wkT = nc.dram_tensor("wkT", [C, JL], BF16, kind="ExternalInput")
    wvT = nc.dram_tensor("wvT", [C, JL], BF16, kind="ExternalInput")
    wpT = nc.dram_tensor("wpT", [JL, C], BF16, kind="ExternalInput")
    out = nc.dram_tensor("out", [T, C], F32, kind="ExternalOutput")
    # bounce buffer for broadcasting softmax reciprocals across partitions
    rcd = nc.dram_tensor("rcd", [NPAIR, NQC, 2, 512], F32)

    xT_r = xT.rearrange("(ci p) t -> p ci t", p=128)
    wq_r = wqT.rearrange("(ci p) j -> p ci j", p=128)
    wk_r = wkT.rearrange("(ci p) j -> p ci j", p=128)
    wv_r = wvT.rearrange("(ci p) j -> p ci j", p=128)
    wp_r = wpT.rearrange("(ji p) c -> p ji c", p=128)

    with tile.TileContext(nc) as tc, ExitStack() as ctx:
        pm = ctx.enter_context(tc.tile_pool(name="pm", bufs=1))
        qkp = ctx.enter_context(tc.tile_pool(name="qkp", bufs=1))
        expp = ctx.enter_context(tc.tile_pool(name="expp", bufs=3))
        cpp = ctx.enter_context(tc.tile_pool(name="cpp", bufs=2))
        rcp = ctx.enter_context(tc.tile_pool(name="rcp", bufs=2))
        bcp = ctx.enter_context(tc.tile_pool(name="bcp", bufs=2))
        stp = ctx.enter_context(tc.tile_pool(name="stp", bufs=2))
        outp = ctx.enter_context(tc.tile_pool(name="outp", bufs=3))
        # PSUM budget (8 banks): fillp 2 + gp 4 + yp 2
        fillp = ctx.enter_context(tc.tile_pool(name="fillp", bufs=2, space="PSUM"))
        gp = ctx.enter_context(tc.tile_pool(name="gp", bufs=2, space="PSUM"))
        yp = ctx.enter_context(tc.tile_pool(name="yp", bufs=2, space="PSUM"))

        x_sb = pm.tile([128, CI, T], BF16, tag="x")
        wq_sb = pm.tile([128, CI, JL], BF16, tag="wq")
        wk_sb = pm.tile([128, CI, JL], BF16, tag="wk")
        wv_sb = pm.tile([128, CI, JL], BF16, tag="wv")
        wp_sb = pm.tile([128, NPAIR, C], BF16, tag="wp")
        # v with a ones column prepended per head (softmax sums land on psum
        # partition 0) plus 64 pad columns so every per-head lhsT reads as
        # [128, 128] (fast weight load path).
        v_sb = pm.tile([128, NT, 8 * VW + 64], BF16, tag="v")
        v_view = v_sb[:, :, 0 : 8 * VW].rearrange("p n (h w) -> p n h w", w=VW)
        qT_all = qkp.tile([128, NPAIR, T], BF16, tag="qT")
        kT_all = qkp.tile([128, NPAIR, T], BF16, tag="kT")
        yT_all = qkp.tile([128, NPAIR, T], BF16, tag="yT")

        ones_col = pm.tile([128, NT, 8, 1], F32)
        nc.vector.memset(ones_col[:], 1.0)
        nc.vector.tensor_copy(v_view[:, :, :, 0:1], ones_col[:])
        nc.vector.memset(v_sb[:, :, 8 * VW : 8 * VW + 64], 0.0)

        # ---- input DMAs, priority order (first q/k chains unblock first) --
        for ci in range(CI):
            nc.sync.dma_start(wq_sb[:, ci, :], wq_r[:, ci, :])
            nc.sync.dma_start(wk_sb[:, ci, :], wk_r[:, ci, :])
            nc.sync.dma_start(x_sb[:, ci, 0:512], xT_r[:, ci, 0:512])
        for ci in range(CI):
            nc.sync.dma_start(wv_sb[:, ci, :], wv_r[:, ci, :])
            nc.sync.dma_start(x_sb[:, ci, 512:1024], xT_r[:, ci, 512:1024])
        nc.sync.dma_start(wp_sb[:], wp_r[:])
        for tch in (2, 3):
            ts = slice(tch * 512, tch * 512 + 512)
            for ci in range(CI):
                nc.sync.dma_start(x_sb[:, ci, ts], xT_r[:, ci, ts])

        # ---- filler machinery: proj/outproj matmuls pad attention gaps ----
        fill_steps = []
        tag_end = {}
        cursor = [0]

        def _consume_to(end):
            end = min(end, len(fill_steps))
            while cursor[0] < end:
                fill_steps[cursor[0]]()
                cursor[0] += 1

        def consume(n):
            _consume_to(cursor[0] + n)

        def consume_until(tag):
            if tag in tag_end:
                _consume_to(tag_end[tag])

        def avail():
            return len(fill_steps) - cursor[0]

        def add_qk_unit(pr, tch):
            ts = slice(tch * 512, tch * 512 + 512)
            for w_sb, dst in ((wq_sb, qT_all), (wk_sb, kT_all)):
                box = {}
                for ci in range(CI):
                    def mm(ci=ci, w_sb=w_sb, box=box):
                        if ci == 0:
                            box["acc"] = fillp.tile([128, 512], F32, tag="fill", name="facc")
                        nc.tensor.matmul(
                            box["acc"][:],
                            w_sb[:, ci, pr * 128 : pr * 128 + 128],
                            x_sb[:, ci, ts],
                            start=(ci == 0),
                            stop=(ci == CI - 1),
                        )
                    fill_steps.append(mm)
                def cp(dst=dst, box=box):
                    nc.vector.tensor_copy(dst[:, pr, ts], box["acc"][:])
                fill_steps.append(cp)
            tag_end[("qk", pr, tch)] = len(fill_steps)

        def add_v_unit(ti):
            tch, tl = divmod(ti, 4)
            base = tch * 512 + tl * 128
            box = {}
            for ci in range(CI):
                def mm(ci=ci, box=box):
                    if ci == 0:
                        box["acc"] = fillp.tile([128, 512], F32, tag="fill", name="facc")
                    nc.tensor.matmul(
                        box["acc"][:],
                        x_sb[:, ci, base : base + 128],
                        wv_sb[:, ci, :],
                        start=(ci == 0),
                        stop=(ci == CI - 1),
                    )
                fill_steps.append(mm)
            def cp(box=box, ti=ti):
                nc.vector.tensor_copy(
                    v_view[:, ti, :, 1:VW],
                    box["acc"][:].rearrange("p (h d) -> p h d", d=D),
                )
            fill_steps.append(cp)
            tag_end[("v", ti)] = len(fill_steps)

        def add_op_unit(ti, cc):
            tss = slice(ti * 128, ti * 128 + 128)
            cs = slice(cc * 512, cc * 512 + 512)
            box = {}
            for ji in range(NPAIR):
                def mm(ji=ji, box=box):
                    if ji == 0:
                        box["acc"] = fillp.tile([128, 512], F32, tag="fill", name="facc")
                    nc.tensor.matmul(
                        box["acc"][:],
                        yT_all[:, ji, tss],
                        wp_sb[:, ji, cs],
                        start=(ji == 0),
                        stop=(ji == NPAIR - 1),
                    )
                fill_steps.append(mm)
            def cpdma(box=box):
                o = outp.tile([128, 512], F32, tag="o")
                nc.vector.tensor_copy(o[:], box["acc"][:])
                nc.sync.dma_start(out[tss, cs], o[:])
            fill_steps.append(cpdma)

        # upfront: deps of the very first attention block
        add_qk_unit(0, 0)
        add_v_unit(0)
        consume(1 << 30)
        for ti in (1, 2, 3):
            add_v_unit(ti)
        for pr in (1, 2, 3):
            add_qk_unit(pr, 0)

        # pace fillers over a horizon of (rest of this round + next round)
        horizon = [0]
        pending = [None]  # deferred normalize of the previous block

        def pace():
            if horizon[0] > 0 and avail() > 0:
                consume(-(-avail() // horizon[0]))
            horizon[0] -= 1

        round_kts = [NPAIR * (4 * qc + 4) for qc in range(NQC)]

        for qc in range(NQC):
            if qc < NQC - 1:
                add_qk_unit(0, qc + 1)
                add_qk_unit(1, qc + 1)
                for ti in range(4 * qc + 4, 4 * qc + 8):
                    add_v_unit(ti)
                add_qk_unit(2, qc + 1)
                add_qk_unit(3, qc + 1)
            nkt = 4 * qc + 4
            horizon[0] = round_kts[qc] + (
                round_kts[qc + 1] if qc + 1 < NQC else 0
            )
            for pr in range(NPAIR):
                consume_until(("qk", pr, qc))
                qs = slice(qc * 512, qc * 512 + 512)
                qlo = qT_all[0:64, pr, :]
                qhi = qT_all[64:128, pr, :]
                klo = kT_all[0:64, pr, :]
                khi = kT_all[64:128, pr, :]
                yA = yp.tile([128, 512], F32, tag="y")
                yB = yp.tile([128, 512], F32, tag="y")

                def emit_pv(kt, e, yA=yA, yB=yB, pr=pr, qc=qc, nkt=nkt):
                    dt = kt - 4 * qc
                    lo = dt * 128 if dt > 0 else 0
                    nc.tensor.matmul(
                        yA[:, lo:512],
                        v_sb[:, kt, 2 * pr * VW : 2 * pr * VW + 128],
                        e[:, 0, lo:512],
                        start=(kt == 0),
                        stop=(kt == nkt - 1),
                    )
                    nc.tensor.matmul(
                        yB[:, lo:512],
                        v_sb[:, kt, (2 * pr + 1) * VW : (2 * pr + 1) * VW + 128],
                        e[:, 1, lo:512],
                        start=(kt == 0),
                        stop=(kt == nkt - 1),
                    )

                prev = None
                for kt in range(nkt):
                    pace()
                    dt = kt - 4 * qc
                    xlo = dt * 128 if dt > 0 else 0
                    ks = slice(kt * 128, kt * 128 + 128)
                    qsx = slice(qc * 512 + xlo, qc * 512 + 512)
                    g = gp.tile([128, 2, 512], F32, tag="g")
                    # the two heads' score matmuls run concurrently on PE
                    # row-groups (K=64 at base partitions 0 and 64)
                    nc.tensor.matmul(
                        g[:, 0, xlo:512], klo[:, ks], qlo[:, qsx],
                        start=True, stop=True,
                    )
                    nc.tensor.matmul(
                        g[:, 1, xlo:512], khi[:, ks], qhi[:, qsx],
                        start=True, stop=True,
                    )
                    e = expp.tile([128, 2, 512], BF16, tag="e")
                    nc.scalar.activation(
                        e[:, :, xlo:512],
                        g[:, :, xlo:512],
                        mybir.ActivationFunctionType.Exp,
                        scale=0.125,
                    )
                    if dt >= 0:
                        # zero the causal triangle (k > q) of the diagonal
                        # block on the otherwise-idle gpsimd engine
                        bs = slice(dt * 128, dt * 128 + 128)
                        for h in (0, 1):
                            nc.gpsimd.affine_select(
                                out=e[:, h, bs],
                                in_=e[:, h, bs],
                                compare_op=mybir.AluOpType.is_ge,
                                fill=0.0,
                                base=0,
                                pattern=[[1, 128]],
                                channel_multiplier=-1,
                            )
                    if prev is not None:
                        consume_until(("v", prev[0]))
                        emit_pv(*prev)
                    prev = (kt, e)
                consume_until(("v", prev[0]))
                emit_pv(*prev)

                # finish the PREVIOUS block's normalize (its reciprocal
                # broadcast DMA had a whole block to land)
                if pending[0] is not None:
                    pending[0]()
                # quick-release PSUM y banks, compute reciprocals, and kick
                # off the partition-broadcast DMA bounce; the multiply and
                # yT repartition run at the NEXT block's trailer
                cp = cpp.tile([VW, 2, 512], F32, tag="cp")
                nc.vector.tensor_copy(cp[:, 0, :], yA[0:VW, :])
                nc.vector.tensor_copy(cp[:, 1, :], yB[0:VW, :])
                rc = rcp.tile([1, 2, 512], F32, tag="rc")
                nc.vector.reciprocal_approx_fast(rc[0:1, 0, :], cp[0:1, 0, :])
                nc.vector.reciprocal_approx_fast(rc[0:1, 1, :], cp[0:1, 1, :])
                nc.sync.dma_start(rcd[pr, qc, :, :], rc[0:1, :, :])
                bc = bcp.tile([VW, 2, 512], F32, tag="bc")
                s = rcd[pr, qc, :, :]
                src = bass.AP(
                    tensor=s.tensor,
                    offset=s.offset,
                    ap=[[0, VW]] + list(s.ap),
                )
                nc.sync.dma_start(bc[:], src)

                def fin(cp=cp, bc=bc, pr=pr, qs=qs):
                    stg = stp.tile([VW, 2, 512], BF16, tag="stg", name="stg")
                    nc.vector.tensor_mul(stg[:], cp[:], bc[:])
                    nc.sync.dma_start(yT_all[0:64, pr, qs], stg[1:VW, 0, :])
                    nc.sync.dma_start(yT_all[64:128, pr, qs], stg[1:VW, 1, :])

                pending[0] = fin
                # this round's outputs become filler work one block later
                # (their yT inputs complete at the next block's trailer)
                if pr == 0 and qc > 0:
                    for ti in range(4 * (qc - 1), 4 * qc):
                        for cc in range(2):
                            add_op_unit(ti, cc)

        pending[0]()
        pending[0] = None
        for ti in range(4 * (NQC - 1), 4 * NQC):
            for cc in range(2):
                add_op_unit(ti, cc)
        consume(1 << 30)

    nc.finalize()
    return nc


def _get_nc():
    global _CACHED_NC
    if _CACHED_NC is None:
        _CACHED_NC = build_nc()
    return _CACHED_NC


def kernel(x, Wq, Wk, Wv, Wp):
    import ml_dtypes
    from concourse.bass_utils import run_bass_kernel_spmd

    bf16 = ml_dtypes.bfloat16
    x = np.asarray(x, dtype=np.float32)
    Wq = np.asarray(Wq, dtype=np.float32)
    Wk = np.asarray(Wk, dtype=np.float32)
    Wv = np.asarray(Wv, dtype=np.float32)
    Wp = np.asarray(Wp, dtype=np.float32)

    nc = _get_nc()

    xT = [np.ascontiguousarray(x[b].T).astype(bf16) for b in range(B)]
    wqT, wkT, wvT, wpT = [], [], [], []
    for hh in range(2):
        js = slice(JL * hh, JL * hh + JL)
        wqT.append(np.ascontiguousarray(Wq[js, :].T).astype(bf16))
        wkT.append(np.ascontiguousarray(Wk[js, :].T).astype(bf16))
        wvT.append(np.ascontiguousarray(Wv[js, :].T).astype(bf16))
        wpT.append(np.ascontiguousarray(Wp[:, js].T).astype(bf16))

    in_maps = []
    for c in range(NCORES):
        b, hh = c // 2, c % 2
        in_maps.append(
            {
                "xT": xT[b],
                "wqT": wqT[hh],
                "wkT": wkT[hh],
                "wvT": wvT[hh],
                "wpT": wpT[hh],
            }
        )

    res = run_bass_kernel_spmd(nc, in_maps, core_ids=list(range(NCORES)))

    out = np.empty((B, T, C), dtype=np.float32)
    for b in range(B):
        out[b] = res.results[2 * b]["out"] + res.results[2 * b + 1]["out"]
    return out


# revision 13
# speedup vs baseline: 1.6324x; 1.0102x over previous
"""Causal self-attention Bass kernel for 8 TRN2 NeuronCores.

Problem: B=4, T=2048, C=1024, H=16 heads, head_dim=64, fp32.
    q = x @ Wq.T ; k = x @ Wk.T ; v = x @ Wv.T          (per head)
    att = softmax(mask(q k^T / 8))
    y = att @ v ; out = y @ Wp.T

Sharding (8 cores): 4-way data parallel over batch x 2-way tensor
parallel over heads. Core c handles batch c//2 and heads 8*(c%2)..+8.
Wq/Wk/Wv column-parallel, Wp row-parallel; the partial outputs of the
two head-halves of each batch are summed on the host (the "all-reduce"
of row-parallel Wp).

Single fused pipeline (no phase barriers): the attention loop runs
qc-outer (q chunks of 512) / head-pair inner / k-tile innermost, and a
filler queue interleaves projection + output-projection matmuls into
the PE bubbles left while ScalarE computes exp.  Everything is bf16
(host-pretransposed and pre-cast), which keeps weight loads on the
fast path and halves input DMA.  Scores for the two heads of a pair
run CONCURRENTLY on PE row-groups (K=64 contraction, lhsT at base
partitions 0/64 -> auto tile_position (0,0)/(64,0)).  Softmax
normalization is moved off the critical path: PSUM y is quick-released
via a copy, then reciprocal (DVE) -> partition_broadcast + multiply
(GpSimd) -> repartition DMA into yT_all.
"""

from contextlib import ExitStack

import numpy as np

import concourse.bass as bass
import concourse.tile as tile
from concourse import bacc, mybir

F32 = mybir.dt.float32
BF16 = mybir.dt.bfloat16

B, T, C, H, D = 4, 2048, 1024, 16, 64
NCORES = 8
JL = 512            # local j dims per core (8 heads * 64)
NPAIR = 4           # local head pairs
CI = C // 128       # 8 c-tiles
NT = T // 128       # 16 t/k tiles
NQC = T // 512      # 4 q chunks
VW = D + 1          # ones column + head dim

_CACHED_NC = None


def build_nc():
    nc = bacc.Bacc(None)

    xT = nc.dram_tensor("xT", [C, T], BF16, kind="ExternalInput")
    wqT = nc.dram_tensor("wqT", [C, JL], BF16, kind="ExternalInput")
    wkT = nc.dram_tensor("wkT", [C, JL], BF16, kind="ExternalInput")
    wvT = nc.dram_tensor("wvT", [C, JL], BF16, kind="ExternalInput")
    wpT = nc.dram_tensor("wpT", [JL, C], BF16, kind="ExternalInput")
    out = nc.dram_tensor("out", [T, C], F32, kind="ExternalOutput")
    # bounce buffer for broadcasting softmax reciprocals across partitions
    rcd = nc.dram_tensor("rcd", [NPAIR, NQC, 2, 512], F32)

    xT_r = xT.rearrange("(ci p) t -> p ci t", p=128)
    wq_r = wqT.rearrange("(ci p) j -> p ci j", p=128)
    wk_r = wkT.rearrange("(ci p) j -> p ci j", p=128)
    wv_r = wvT.rearrange("(ci p) j -> p ci j", p=128)
    wp_r = wpT.rearrange("(ji p) c -> p ji c", p=128)

    with tile.TileContext(nc) as tc, ExitStack() as ctx:
        pm = ctx.enter_context(tc.tile_pool(name="pm", bufs=1))
        qkp = ctx.enter_context(tc.tile_pool(name="qkp", bufs=1))
        expp = ctx.enter_context(tc.tile_pool(name="expp", bufs=5))
        cpp = ctx.enter_context(tc.tile_pool(name="cpp", bufs=2))
        rcp = ctx.enter_context(tc.tile_pool(name="rcp", bufs=2))
        bcp = ctx.enter_context(tc.tile_pool(name="bcp", bufs=2))
        stp = ctx.enter_context(tc.tile_pool(name="stp", bufs=2))
        outp = ctx.enter_context(tc.tile_pool(name="outp", bufs=3))
        # PSUM budget (8 banks): fillp 2 + gp 4 + yp 2
        fillp = ctx.enter_context(tc.tile_pool(name="fillp", bufs=2, space="PSUM"))
        gp = ctx.enter_context(tc.tile_pool(name="gp", bufs=2, space="PSUM"))
        yp = ctx.enter_context(tc.tile_pool(name="yp", bufs=2, space="PSUM"))

        x_sb = pm.tile([128, CI, T], BF16, tag="x")
        wq_sb = pm.tile([128, CI, JL], BF16, tag="wq")
        wk_sb = pm.tile([128, CI, JL], BF16, tag="wk")
        wv_sb = pm.tile([128, CI, JL], BF16, tag="wv")
        wp_sb = pm.tile([128, NPAIR, C], BF16, tag="wp")
        # v with a ones column prepended per head (softmax sums land on psum
        # partition 0) plus 64 pad columns so every per-head lhsT reads as
        # [128, 128] (fast weight load path).
        v_sb = pm.tile([128, NT, 8 * VW + 64], BF16, tag="v")
        v_view = v_sb[:, :, 0 : 8 * VW].rearrange("p n (h w) -> p n h w", w=VW)
        qT_all = qkp.tile([128, NPAIR, T], BF16, tag="qT")
        kT_all = qkp.tile([128, NPAIR, T], BF16, tag="kT")
        yT_all = qkp.tile([128, NPAIR, T], BF16, tag="yT")

        # ---- input DMAs: few big transfers, triggers spread across engine
        # queues so they issue in parallel (Sync alone takes ~630ns/trigger)
        nc.sync.dma_start(wq_sb[:, 0:4, :], wq_r[:, 0:4, :])
        nc.scalar.dma_start(x_sb[:, 0:4, 0:512], xT_r[:, 0:4, 0:512])
        nc.gpsimd.dma_start(wk_sb[:, 0:4, :], wk_r[:, 0:4, :])
        nc.sync.dma_start(wq_sb[:, 4:8, :], wq_r[:, 4:8, :])
        nc.scalar.dma_start(x_sb[:, 4:8, 0:512], xT_r[:, 4:8, 0:512])
        nc.gpsimd.dma_start(wk_sb[:, 4:8, :], wk_r[:, 4:8, :])
        nc.sync.dma_start(wv_sb[:], wv_r[:])
        nc.scalar.dma_start(x_sb[:, :, 512:1024], xT_r[:, :, 512:1024])

        ones_col = pm.tile([128, NT, 8, 1], F32)
        nc.gpsimd.memset(ones_col[:], 1.0)
        nc.gpsimd.tensor_copy(v_view[:, :, :, 0:1], ones_col[:])
        nc.gpsimd.memset(v_sb[:, :, 8 * VW : 8 * VW + 64], 0.0)

        # ---- filler machinery: proj/outproj matmuls pad attention gaps ----
        fill_steps = []
        tag_end = {}
        cursor = [0]

        def _consume_to(end):
            end = min(end, len(fill_steps))
            while cursor[0] < end:
                fill_steps[cursor[0]]()
                cursor[0] += 1

        def consume(n):
            _consume_to(cursor[0] + n)

        def consume_until(tag):
            if tag in tag_end:
                _consume_to(tag_end[tag])

        def avail():
            return len(fill_steps) - cursor[0]

        def add_qk_unit(pr, tch):
            ts = slice(tch * 512, tch * 512 + 512)
            for w_sb, dst in ((wq_sb, qT_all), (wk_sb, kT_all)):
                box = {}
                for ci in range(CI):
                    def mm(ci=ci, w_sb=w_sb, box=box):
                        if ci == 0:
                            box["acc"] = fillp.tile([128, 512], F32, tag="fill", name="facc")
                        nc.tensor.matmul(
                            box["acc"][:],
                            w_sb[:, ci, pr * 128 : pr * 128 + 128],
                            x_sb[:, ci, ts],
                            start=(ci == 0),
                            stop=(ci == CI - 1),
                        )
                    fill_steps.append(mm)
                def cp(dst=dst, box=box):
                    nc.vector.tensor_copy(dst[:, pr, ts], box["acc"][:])
                fill_steps.append(cp)
            tag_end[("qk", pr, tch)] = len(fill_steps)

        def add_v_unit(ti):
            tch, tl = divmod(ti, 4)
            base = tch * 512 + tl * 128
            box = {}
            for ci in range(CI):
                def mm(ci=ci, box=box):
                    if ci == 0:
                        box["acc"] = fillp.tile([128, 512], F32, tag="fill", name="facc")
                    nc.tensor.matmul(
                        box["acc"][:],
                        x_sb[:, ci, base : base + 128],
                        wv_sb[:, ci, :],
                        start=(ci == 0),
                        stop=(ci == CI - 1),
                    )
                fill_steps.append(mm)
            def cp(box=box, ti=ti):
                nc.vector.tensor_copy(
                    v_view[:, ti, :, 1:VW],
                    box["acc"][:].rearrange("p (h d) -> p h d", d=D),
                )
            fill_steps.append(cp)
            tag_end[("v", ti)] = len(fill_steps)

        def add_op_unit(ti, cc):
            tss = slice(ti * 128, ti * 128 + 128)
            cs = slice(cc * 512, cc * 512 + 512)
            box = {}
            for ji in range(NPAIR):
                def mm(ji=ji, box=box):
                    if ji == 0:
                        box["acc"] = fillp.tile([128, 512], F32, tag="fill", name="facc")
                    nc.tensor.matmul(
                        box["acc"][:],
                        yT_all[:, ji, tss],
                        wp_sb[:, ji, cs],
                        start=(ji == 0),
                        stop=(ji == NPAIR - 1),
                    )
                fill_steps.append(mm)
            def cpdma(box=box):
                o = outp.tile([128, 512], F32, tag="o")
                nc.vector.tensor_copy(o[:], box["acc"][:])
                nc.sync.dma_start(out[tss, cs], o[:])
            fill_steps.append(cpdma)

        # upfront: deps of the very first attention block
        add_qk_unit(0, 0)
        add_v_unit(0)
        consume(1 << 30)
        for ti in (1, 2, 3):
            add_v_unit(ti)
        for pr in (1, 2, 3):
            add_qk_unit(pr, 0)

        # pace fillers over a horizon of (rest of this round + next round)
        horizon = [0]
        pending = [None]  # deferred normalize of the previous block

        def pace():
            if horizon[0] > 0 and avail() > 0:
                consume(-(-avail() // horizon[0]))
            horizon[0] -= 1

        round_kts = [NPAIR * (4 * qc + 4) for qc in range(NQC)]

        for qc in range(NQC):
            if qc == 1:
                # late inputs: issue once startup-critical transfers are done
                # (gpsimd reaches these triggers ~mid round 0)
                nc.gpsimd.dma_start(wp_sb[:], wp_r[:])
                nc.gpsimd.dma_start(x_sb[:, :, 1024:1536], xT_r[:, :, 1024:1536])
                nc.gpsimd.dma_start(x_sb[:, :, 1536:2048], xT_r[:, :, 1536:2048])
            if qc < NQC - 1:
                add_qk_unit(0, qc + 1)
                add_qk_unit(1, qc + 1)
                for ti in range(4 * qc + 4, 4 * qc + 8):
                    add_v_unit(ti)
                add_qk_unit(2, qc + 1)
                add_qk_unit(3, qc + 1)
            nkt = 4 * qc + 4
            horizon[0] = round_kts[qc] + (
                round_kts[qc + 1] if qc + 1 < NQC else 0
            )
            for pr in range(NPAIR):
                consume_until(("qk", pr, qc))
                qs = slice(qc * 512, qc * 512 + 512)
                qlo = qT_all[0:64, pr, :]
                qhi = qT_all[64:128, pr, :]
                klo = kT_all[0:64, pr, :]
                khi = kT_all[64:128, pr, :]
                yA = yp.tile([128, 512], F32, tag="y")
                yB = yp.tile([128, 512], F32, tag="y")

                def emit_pv(kt, e, yA=yA, yB=yB, pr=pr, qc=qc, nkt=nkt):
                    dt = kt - 4 * qc
                    lo = dt * 128 if dt > 0 else 0
                    nc.tensor.matmul(
                        yA[:, lo:512],
                        v_sb[:, kt, 2 * pr * VW : 2 * pr * VW + 128],
                        e[:, 0, lo:512],
                        start=(kt == 0),
                        stop=(kt == nkt - 1),
                    )
                    nc.tensor.matmul(
                        yB[:, lo:512],
                        v_sb[:, kt, (2 * pr + 1) * VW : (2 * pr + 1) * VW + 128],
                        e[:, 1, lo:512],
                        start=(kt == 0),
                        stop=(kt == nkt - 1),
                    )

                pvq = []
                for kt in range(nkt):
                    pace()
                    dt = kt - 4 * qc
                    xlo = dt * 128 if dt > 0 else 0
                    ks = slice(kt * 128, kt * 128 + 128)
                    qsx = slice(qc * 512 + xlo, qc * 512 + 512)
                    g = gp.tile([128, 2, 512], F32, tag="g")
                    # the two heads' score matmuls run concurrently on PE
                    # row-groups (K=64 at base partitions 0 and 64)
                    nc.tensor.matmul(
                        g[:, 0, xlo:512], klo[:, ks], qlo[:, qsx],
                        start=True, stop=True,
                    )
                    nc.tensor.matmul(
                        g[:, 1, xlo:512], khi[:, ks], qhi[:, qsx],
                        start=True, stop=True,
                    )
                    e = expp.tile([128, 2, 512], BF16, tag="e")
                    nc.scalar.activation(
                        e[:, :, xlo:512],
                        g[:, :, xlo:512],
                        mybir.ActivationFunctionType.Exp,
                        scale=0.125,
                    )
                    if dt >= 0:
                        # zero the causal triangle (k > q) of the diagonal
                        # block on the otherwise-idle gpsimd engine
                        bs = slice(dt * 128, dt * 128 + 128)
                        for h in (0, 1):
                            nc.gpsimd.affine_select(
                                out=e[:, h, bs],
                                in_=e[:, h, bs],
                                compare_op=mybir.AluOpType.is_ge,
                                fill=0.0,
                                base=0,
                                pattern=[[1, 128]],
                                channel_multiplier=-1,
                            )
                    pvq.append((kt, e))
                    if len(pvq) > 2:
                        ktp, ep = pvq.pop(0)
                        consume_until(("v", ktp))
                        emit_pv(ktp, ep)
                for ktp, ep in pvq:
                    consume_until(("v", ktp))
                    emit_pv(ktp, ep)
                pvq = []

                # finish the PREVIOUS block's normalize (its reciprocal
                # broadcast DMA had a whole block to land)
                if pending[0] is not None:
                    pending[0]()
                # quick-release PSUM y banks, compute reciprocals, and kick
                # off the partition-broadcast DMA bounce; the multiply and
                # yT repartition run at the NEXT block's trailer
                cp = cpp.tile([VW, 2, 512], F32, tag="cp")
                nc.vector.tensor_copy(cp[:, 0, :], yA[0:VW, :])
                nc.vector.tensor_copy(cp[:, 1, :], yB[0:VW, :])
                rc = rcp.tile([1, 2, 512], F32, tag="rc")
                nc.vector.reciprocal_approx_fast(rc[0:1, 0, :], cp[0:1, 0, :])
                nc.vector.reciprocal_approx_fast(rc[0:1, 1, :], cp[0:1, 1, :])
                nc.sync.dma_start(rcd[pr, qc, :, :], rc[0:1, :, :])
                bc = bcp.tile([VW, 2, 512], F32, tag="bc")
                s = rcd[pr, qc, :, :]
                src = bass.AP(
                    tensor=s.tensor,
                    offset=s.offset,
                    ap=[[0, VW]] + list(s.ap),
                )
                nc.sync.dma_start(bc[:], src)

                def fin(cp=cp, bc=bc, pr=pr, qs=qs):
                    stg = stp.tile([VW, 2, 512], BF16, tag="stg", name="stg")
                    nc.vector.tensor_mul(stg[:], cp[:], bc[:])
                    nc.sync.dma_start(yT_all[0:64, pr, qs], stg[1:VW, 0, :])
                    nc.sync.dma_start(yT_all[64:128, pr, qs], stg[1:VW, 1, :])

                pending[0] = fin
                # this round's outputs become filler work one block later
                # (their yT inputs complete at the next block's trailer)
                if pr == 0 and qc > 0:
                    for ti in range(4 * (qc - 1), 4 * qc):
                        for cc in range(2):
                            add_op_unit(ti, cc)

        pending[0]()
        pending[0] = None
        for ti in range(4 * (NQC - 1), 4 * NQC):
            for cc in range(2):
                add_op_unit(ti, cc)
        consume(1 << 30)

    nc.finalize()
    return nc


def _get_nc():
    global _CACHED_NC
    if _CACHED_NC is None:
        _CACHED_NC = build_nc()
    return _CACHED_NC


def kernel(x, Wq, Wk, Wv, Wp):
    import ml_dtypes
    from concourse.bass_utils import run_bass_kernel_spmd

    bf16 = ml_dtypes.bfloat16
    x = np.asarray(x, dtype=np.float32)
    Wq = np.asarray(Wq, dtype=np.float32)
    Wk = np.asarray(Wk, dtype=np.float32)
    Wv = np.asarray(Wv, dtype=np.float32)
    Wp = np.asarray(Wp, dtype=np.float32)

    nc = _get_nc()

    xT = [np.ascontiguousarray(x[b].T).astype(bf16) for b in range(B)]
    wqT, wkT, wvT, wpT = [], [], [], []
    for hh in range(2):
        js = slice(JL * hh, JL * hh + JL)
        wqT.append(np.ascontiguousarray(Wq[js, :].T).astype(bf16))
        wkT.append(np.ascontiguousarray(Wk[js, :].T).astype(bf16))
        wvT.append(np.ascontiguousarray(Wv[js, :].T).astype(bf16))
        wpT.append(np.ascontiguousarray(Wp[:, js].T).astype(bf16))

    in_maps = []
    for c in range(NCORES):
        b, hh = c // 2, c % 2
        in_maps.append(
            {
                "xT": xT[b],
                "wqT": wqT[hh],
                "wkT": wkT[hh],
                "wvT": wvT[hh],
                "wpT": wpT[hh],
            }
        )

    res = run_bass_kernel_spmd(nc, in_maps, core_ids=list(range(NCORES)))

    out = np.empty((B, T, C), dtype=np.float32)
    for b in range(B):
        out[b] = res.results[2 * b]["out"] + res.results[2 * b + 1]["out"]
    return out
